# revision 7
# baseline (speedup 1.0000x reference)
"""DIMKT scan kernel for 8x Trainium2 NeuronCores (Bass/Tile).

Data-parallel over batch (64 rows/core). Host packs derived weight tables
(weight-side transforms only); device gathers per-token rows, transposes them
into PSUM as gate accumulation bases, and runs the sequential scan with
5 small matmuls + 2 strided sigmoids (tanh(x) = 2*sigmoid(2x) - 1) per step.
y_t = sigmoid(dot(x_{t+1}, h_t)) via a ones-column matmul batched per chunk.
"""
import numpy as np

B, S, D = 512, 500, 128
NQ, NC, NQD, NCD = 10000, 500, 100, 100
NCORES = 8
BC = B // NCORES          # 64 batch rows per core
CH = 4                    # timesteps per chunk
NSTEP = S - 1             # 499 scan steps
NCHUNK = (NSTEP + CH - 1) // CH   # 125 (last chunk has 3 steps)
XTOK = S * BC             # 32000 x tokens per core
GTOK = 128                # tokens per gather group
NGRP = XTOK // GTOK       # 250 groups

_cache = {}


def _host_pack(Eq, Ec, Eqd, Ecd, Ecorr, Wx, bx, Wsdf1, bsdf1, Wsdf2, bsdf2,
               Wpka1, bpka1, Wpka2, bpka2, Wki, bki):
    f32 = np.float32
    Wx0, Wx1, Wx2, Wx3 = (np.asarray(Wx[i * D:(i + 1) * D], f32) for i in range(4))
    T_q = np.asarray(Eq, f32) @ Wx0
    T_c = np.asarray(Ec, f32) @ Wx1 + np.asarray(bx, f32)
    A = np.asarray(Eqd, f32) @ Wx2            # [100,128]
    Bt = np.asarray(Ecd, f32) @ Wx3           # [100,128]
    T_qdcd = (A[:, None, :] + Bt[None, :, :]).reshape(NQD * NCD, D).astype(f32)
    # COMB[(qd*200 + cd*2 + co)] rows: [ki_part | pka1_part | 2*pka2_part]
    KI_qd = np.asarray(Eqd, f32) @ np.asarray(Wki[2 * D:3 * D], f32)
    KI_cd = np.asarray(Ecd, f32) @ np.asarray(Wki[3 * D:4 * D], f32)
    KI_co = np.asarray(Ecorr, f32) @ np.asarray(Wki[D:2 * D], f32) + np.asarray(bki, f32)
    P1_co = np.asarray(Ecorr, f32) @ np.asarray(Wpka1[D:2 * D], f32) + np.asarray(bpka1, f32)
    P2_co = 2.0 * (np.asarray(Ecorr, f32) @ np.asarray(Wpka2[D:2 * D], f32) + np.asarray(bpka2, f32))
    ki = (KI_qd[:, None, None, :] + KI_cd[None, :, None, :] + KI_co[None, None, :, :])
    ki = ki.reshape(NQD * NCD * 2, D)
    p1 = np.broadcast_to(P1_co[None, None, :, :], (NQD, NCD, 2, D)).reshape(-1, D)
    p2 = np.broadcast_to(P2_co[None, None, :, :], (NQD, NCD, 2, D)).reshape(-1, D)
    COMB = np.concatenate([ki, p1, p2], axis=1).astype(f32)   # [20000, 384]
    return dict(
        T_q=np.ascontiguousarray(T_q, f32),
        T_c=np.ascontiguousarray(T_c, f32),
        T_qdcd=np.ascontiguousarray(T_qdcd, f32),
        COMB=np.ascontiguousarray(COMB, f32),
        Wsdf1p=np.ascontiguousarray(Wsdf1, f32),          # +Wsdf1 (x side)
        Wsdf2p2=np.ascontiguousarray(2.0 * Wsdf2, f32),   # +2*Wsdf2 (x side)
        W1n=np.ascontiguousarray(-np.asarray(Wsdf1, f32)),
        W2n2=np.ascontiguousarray(-2.0 * np.asarray(Wsdf2, f32)),
        Wk1=np.ascontiguousarray(Wki[0:D], f32),
        Wp1=np.ascontiguousarray(Wpka1[0:D], f32),
        Wp2x2=np.ascontiguousarray(2.0 * np.asarray(Wpka2[0:D], f32)),
    )


def _group_idx(arr_sb):   # [nsteps, BC] step-major -> [128, NGRP] int32 (pad 0)
    flat = arr_sb.reshape(-1)
    pad = NGRP * GTOK - flat.shape[0]
    if pad:
        flat = np.concatenate([flat, np.zeros(pad, flat.dtype)])
    return np.ascontiguousarray(flat.reshape(NGRP, GTOK).T.astype(np.int32))


def _build_program():
    import concourse.bacc as bacc
    import concourse.bass as bass
    import concourse.mybir as mybir
    from concourse.tile import TileContext
    from concourse.masks import make_identity

    f32 = mybir.dt.float32
    Alu = mybir.AluOpType
    Act = mybir.ActivationFunctionType
    nc = bacc.Bacc("TRN2", target_bir_lowering=False, debug=False,
                   num_devices=NCORES, num_swdge_queues=4)

    dram = {}
    for nm, shape, dt in [
        ("T_q", (NQ, D), f32), ("T_c", (NC, D), f32), ("T_qdcd", (NQD * NCD, D), f32),
        ("COMB", (NQD * NCD * 2, 3 * D), f32),
        ("Wsdf1p", (D, D), f32), ("Wsdf2p2", (D, D), f32), ("W1n", (D, D), f32),
        ("W2n2", (D, D), f32), ("Wk1", (D, D), f32), ("Wp1", (D, D), f32),
        ("Wp2x2", (D, D), f32), ("h0T", (D, BC), f32),
        ("qidx", (128, NGRP), mybir.dt.int32), ("cidx", (128, NGRP), mybir.dt.int32),
        ("qdcdidx", (128, NGRP), mybir.dt.int32), ("combidx", (128, NGRP), mybir.dt.int32),
    ]:
        dram[nm] = nc.dram_tensor(nm, shape, dt, kind="ExternalInput")
    t_y = nc.dram_tensor("y", (NCHUNK * CH * BC,), f32, kind="ExternalOutput")

    def gather(out_ap, table, idx_col, queue, accum=False):
        inst = nc.gpsimd.indirect_dma_start(
            out=out_ap, out_offset=None, in_=dram[table].ap(),
            in_offset=bass.IndirectOffsetOnAxis(ap=idx_col, axis=0),
            compute_op=Alu.add if accum else Alu.bypass,
        )
        inst.ins.queue = f"qPoolDynamic{queue or ''}"
        return inst

    with TileContext(nc) as tc:
        with (
            tc.tile_pool(name="const", bufs=1) as cpool,
            tc.tile_pool(name="gath", bufs=3) as gpool,
            tc.tile_pool(name="xt", bufs=3) as xtpool,
            tc.tile_pool(name="step", bufs=3) as spool,
            tc.tile_pool(name="hpool", bufs=3) as hpool,
            tc.tile_pool(name="ppsum", bufs=2, space="PSUM") as ppool,
            tc.tile_pool(name="xpsum", bufs=2, space="PSUM") as xppool,
        ):
            ident = cpool.tile([128, 128], f32)
            make_identity(nc, ident)
            ones_col = cpool.tile([128, 1], f32)
            nc.vector.memset(ones_col[:], 1.0)
            w_sb = {}
            for nm in ["Wsdf1p", "Wsdf2p2", "W1n", "W2n2", "Wk1", "Wp1", "Wp2x2"]:
                w_sb[nm] = cpool.tile([D, D], f32, name=nm, tag=nm)
                nc.sync.dma_start(out=w_sb[nm][:], in_=dram[nm].ap())
            idx_sb = {}
            for nm in ["qidx", "cidx", "qdcdidx", "combidx"]:
                idx_sb[nm] = cpool.tile([128, NGRP], mybir.dt.int32, name=nm, tag=nm)
                nc.sync.dma_start(out=idx_sb[nm][:], in_=dram[nm].ap())
            h = hpool.tile([D, BC], f32, tag="h")
            nc.sync.dma_start(out=h[:], in_=dram["h0T"].ap())

            # deferred y state: (prod_tile, cp_base_ap, h_at_boundary, nst_prev, k_prev)
            pending = None

            for k in range(NCHUNK):
                nst = min(CH, NSTEP - k * CH)
                g0 = 2 * k
                # ---- gathers (token-major rows); one tile per group so each
                # consumer waits on exactly one DMA-queue proc ----
                xgs, cgs = [], []
                for g in range(2):
                    xg = gpool.tile([128, D], f32, tag=f"xg{g}")
                    gather(xg[:], "T_q", idx_sb["qidx"][:, g0 + g:g0 + g + 1], g % 2)
                    gather(xg[:], "T_c", idx_sb["cidx"][:, g0 + g:g0 + g + 1], g % 2, accum=True)
                    gather(xg[:], "T_qdcd", idx_sb["qdcdidx"][:, g0 + g:g0 + g + 1], g % 2, accum=True)
                    xgs.append(xg)
                    cg = gpool.tile([128, 3 * D], f32, tag=f"cg{g}")
                    gather(cg[:], "COMB", idx_sb["combidx"][:, g0 + g:g0 + g + 1], 2 + (g % 2))
                    cgs.append(cg)

                # ---- x^T via PE transpose -> psum -> sbuf ----
                xps = xppool.tile([128, 2 * D], f32, tag="xps")
                for g in range(2):
                    nc.tensor.transpose(out=xps[:, g * D:(g + 1) * D],
                                        in_=xgs[g][:], identity=ident[:])
                xT = xtpool.tile([128, 2 * D], f32, tag="xT")
                nc.vector.tensor_copy(xT[:], xps[:])

                # ---- flush previous chunk's boundary prod + y ----
                if pending is not None:
                    pprod, pct, pco, pca, ph, pnst, pk = pending
                    nc.gpsimd.tensor_tensor(out=pprod[:, (CH - 1) * 64:CH * 64],
                                            in0=ph[:], in1=xT[:, 0:64], op=Alu.mult)
                    nc.tensor.matmul(bass.AP(pct, pco + 1280, [[pca[0][0], 1], [1, 64 * pnst]]),
                                     ones_col[:], pprod[:, 0:64 * pnst],
                                     start=False, stop=True, skip_group_check=True)
                    ysb = spool.tile([1, 256], f32, tag="ysb")
                    nc.scalar.activation(ysb[:1, 0:64 * pnst],
                                         bass.AP(pct, pco + 1280, [[pca[0][0], 1], [1, 64 * pnst]]),
                                         Act.Sigmoid)
                    nc.sync.dma_start(out=t_y.ap()[pk * CH * BC: pk * CH * BC + 64 * pnst],
                                      in_=ysb[:1, 0:64 * pnst])
                    pending = None

                # ---- chunk psum: bankA = sdf1|sdf2', bankB = ki|pka1, bankC = pka2'|y ----
                cp = ppool.tile([128, 3 * 512], f32, tag="cp")
                base = cp[:]
                ct, co, ca = base.tensor, base.offset, base.ap

                def cps(col0, ncols):
                    return bass.AP(ct, co + col0, [[ca[0][0], 128], [1, ncols]])

                for g in range(2):   # ki bases -> bankB cols 0..255 (abs 512..767)
                    nc.tensor.matmul(cps(512 + g * 128, 128),
                                     cgs[g][:, 0:D], ident[:],
                                     start=(g == 0), stop=False,
                                     is_transpose=True, skip_group_check=True)
                for g in range(2):   # pka1 bases -> bankB cols 256..511
                    nc.tensor.matmul(cps(768 + g * 128, 128),
                                     cgs[g][:, D:2 * D], ident[:],
                                     start=False, stop=False,
                                     is_transpose=True, skip_group_check=True)
                for g in range(2):   # pka2' bases -> bankC cols 0..255
                    nc.tensor.matmul(cps(1024 + g * 128, 128),
                                     cgs[g][:, 2 * D:3 * D], ident[:],
                                     start=(g == 0), stop=False,
                                     is_transpose=True, skip_group_check=True)
                # x side of sdf gates -> bankA
                nc.tensor.matmul(cps(0, 256), w_sb["Wsdf1p"][:], xT[:],
                                 start=True, stop=False, skip_group_check=True)
                nc.tensor.matmul(cps(256, 256), w_sb["Wsdf2p2"][:], xT[:],
                                 start=False, stop=False, skip_group_check=True)

                prod = spool.tile([128, 256], f32, tag="prod")

                for s in range(nst):
                    nc.tensor.matmul(cps(0 + s * 64, 64), w_sb["W1n"][:], h[:],
                                     start=False, stop=False, skip_group_check=True)
                    nc.tensor.matmul(cps(256 + s * 64, 64), w_sb["W2n2"][:], h[:],
                                     start=False, stop=False, skip_group_check=True)
                    nc.tensor.matmul(cps(512 + s * 64, 64), w_sb["Wk1"][:], h[:],
                                     start=False, stop=False, skip_group_check=True)
                    gates1 = spool.tile([128, 192], f32, tag="gates1")
                    a1src = bass.AP(ct, co + s * 64, [[ca[0][0], 128], [256, 3], [1, 64]])
                    a1dst = gates1[:].rearrange("p (a b) -> p a b", b=64)
                    nc.scalar.activation(a1dst, a1src, Act.Sigmoid)
                    s1, s2p, gam = gates1[:, 0:64], gates1[:, 64:128], gates1[:, 128:192]
                    m = spool.tile([128, 64], f32, tag="m")
                    nc.vector.scalar_tensor_tensor(out=m[:], in0=s2p, scalar=2.0, in1=s1,
                                                   op0=Alu.mult, op1=Alu.mult)
                    sdf = spool.tile([128, 64], f32, tag="sdf")
                    nc.vector.tensor_tensor(out=sdf[:], in0=m[:], in1=s1, op=Alu.subtract)
                    nc.tensor.matmul(cps(768 + s * 64, 64), w_sb["Wp1"][:], sdf[:],
                                     start=False, stop=False, skip_group_check=True)
                    nc.tensor.matmul(cps(1024 + s * 64, 64), w_sb["Wp2x2"][:], sdf[:],
                                     start=False, stop=False, skip_group_check=True)
                    gates2 = spool.tile([128, 128], f32, tag="gates2")
                    a2src = bass.AP(ct, co + 768 + s * 64, [[ca[0][0], 128], [256, 2], [1, 64]])
                    a2dst = gates2[:].rearrange("p (a b) -> p a b", b=64)
                    nc.scalar.activation(a2dst, a2src, Act.Sigmoid)
                    p1, p2p = gates2[:, 0:64], gates2[:, 64:128]
                    m2 = spool.tile([128, 64], f32, tag="m2")
                    nc.vector.scalar_tensor_tensor(out=m2[:], in0=p2p, scalar=2.0, in1=p1,
                                                   op0=Alu.mult, op1=Alu.mult)
                    pka = spool.tile([128, 64], f32, tag="pka")
                    nc.vector.tensor_tensor(out=pka[:], in0=m2[:], in1=p1, op=Alu.subtract)
                    # h' = gam*h + (1-gam)*pka
                    gamc = spool.tile([128, 64], f32, tag="gamc")
                    nc.gpsimd.tensor_scalar(out=gamc[:], in0=gam, scalar1=-1.0, scalar2=1.0,
                                            op0=Alu.mult, op1=Alu.add)
                    g1 = spool.tile([128, 64], f32, tag="g1")
                    nc.vector.tensor_tensor(out=g1[:], in0=gam, in1=h[:], op=Alu.mult)
                    u = spool.tile([128, 64], f32, tag="u")
                    nc.gpsimd.tensor_tensor(out=u[:], in0=gamc[:], in1=pka[:], op=Alu.mult)
                    hn = hpool.tile([D, BC], f32, tag="h")
                    nc.vector.tensor_tensor(out=hn[:], in0=g1[:], in1=u[:], op=Alu.add)
                    h = hn
                    if s < nst - 1 or k == NCHUNK - 1:
                        nc.gpsimd.tensor_tensor(out=prod[:, s * 64:(s + 1) * 64],
                                                in0=h[:], in1=xT[:, (s + 1) * 64:(s + 2) * 64],
                                                op=Alu.mult)

                if k == NCHUNK - 1:
                    nc.tensor.matmul(bass.AP(ct, co + 1280, [[ca[0][0], 1], [1, 64 * nst]]),
                                     ones_col[:], prod[:, 0:64 * nst],
                                     start=False, stop=True, skip_group_check=True)
                    ysb = spool.tile([1, 256], f32, tag="ysb")
                    nc.scalar.activation(ysb[:1, 0:64 * nst],
                                         bass.AP(ct, co + 1280, [[ca[0][0], 1], [1, 64 * nst]]),
                                         Act.Sigmoid)
                    nc.sync.dma_start(out=t_y.ap()[k * CH * BC: k * CH * BC + 64 * nst],
                                      in_=ysb[:1, 0:64 * nst])
                else:
                    pending = (prod, ct, co, ca, h, nst, k)
    nc.compile()
    return nc


def kernel(**inputs):
    from concourse.bass_utils import run_bass_kernel_spmd

    w = _host_pack(**{k: np.asarray(inputs[k]) for k in
                      ["Eq", "Ec", "Eqd", "Ecd", "Ecorr", "Wx", "bx", "Wsdf1", "bsdf1",
                       "Wsdf2", "bsdf2", "Wpka1", "bpka1", "Wpka2", "bpka2", "Wki", "bki"]})
    q = np.asarray(inputs["question_seq"])
    c = np.asarray(inputs["concept_seq"])
    qd = np.asarray(inputs["question_diff_seq"])
    cd = np.asarray(inputs["concept_diff_seq"])
    co = np.asarray(inputs["correct_seq"])
    h0 = np.asarray(inputs["h0"], np.float32)
    qdcd = (qd * NCD + cd).astype(np.int64)
    comb = (qd * (NCD * 2) + cd * 2 + co).astype(np.int64)

    if "nc" not in _cache:
        _cache["nc"] = _build_program()
    nc = _cache["nc"]

    in_maps = []
    for core in range(NCORES):
        rows = slice(core * BC, (core + 1) * BC)
        m = dict(w)
        m["h0T"] = np.ascontiguousarray(h0[rows].T)
        m["qidx"] = _group_idx(q[rows].T)          # [S, BC] step-major
        m["cidx"] = _group_idx(c[rows].T)
        m["qdcdidx"] = _group_idx(qdcd[rows].T)
        m["combidx"] = _group_idx(comb[rows].T[:NSTEP])
        in_maps.append(m)

    global _last_in_maps
    _last_in_maps = in_maps
    res = run_bass_kernel_spmd(nc, in_maps, list(range(NCORES)))
    y = np.zeros((B, S), np.float32)
    for core in range(NCORES):
        yd = res.results[core]["y"][:NSTEP * BC].reshape(NSTEP, BC)
        y[core * BC:(core + 1) * BC, :NSTEP] = yd.T
    return y



# revision 8
# speedup vs baseline: 1.0982x; 1.0982x over previous
"""DIMKT scan kernel for 8x Trainium2 NeuronCores (Bass/Tile).

Data-parallel over batch (64 rows/core). Host packs derived weight tables
(weight-side transforms only); device gathers per-token rows, transposes them
into PSUM as gate accumulation bases, and runs the sequential scan with
5 small matmuls + 2 strided sigmoids (tanh(x) = 2*sigmoid(2x) - 1) per step.
y_t = sigmoid(dot(x_{t+1}, h_t)) via a ones-column matmul batched per chunk.
"""
import numpy as np

B, S, D = 512, 500, 128
NQ, NC, NQD, NCD = 10000, 500, 100, 100
NCORES = 8
BC = B // NCORES          # 64 batch rows per core
CH = 4                    # timesteps per chunk
NSTEP = S - 1             # 499 scan steps
NCHUNK = (NSTEP + CH - 1) // CH   # 125 (last chunk has 3 steps)
XTOK = S * BC             # 32000 x tokens per core
GTOK = 128                # tokens per gather group
NGRP = XTOK // GTOK       # 250 groups

_cache = {}


def _host_pack(Eq, Ec, Eqd, Ecd, Ecorr, Wx, bx, Wsdf1, bsdf1, Wsdf2, bsdf2,
               Wpka1, bpka1, Wpka2, bpka2, Wki, bki):
    f32 = np.float32
    Wx0, Wx1, Wx2, Wx3 = (np.asarray(Wx[i * D:(i + 1) * D], f32) for i in range(4))
    T_q = np.asarray(Eq, f32) @ Wx0
    T_c = np.asarray(Ec, f32) @ Wx1 + np.asarray(bx, f32)
    A = np.asarray(Eqd, f32) @ Wx2            # [100,128]
    Bt = np.asarray(Ecd, f32) @ Wx3           # [100,128]
    T_qdcd = (A[:, None, :] + Bt[None, :, :]).reshape(NQD * NCD, D).astype(f32)
    # COMB[(qd*200 + cd*2 + co)] rows: [ki_part | pka1_part | 2*pka2_part]
    KI_qd = np.asarray(Eqd, f32) @ np.asarray(Wki[2 * D:3 * D], f32)
    KI_cd = np.asarray(Ecd, f32) @ np.asarray(Wki[3 * D:4 * D], f32)
    KI_co = np.asarray(Ecorr, f32) @ np.asarray(Wki[D:2 * D], f32) + np.asarray(bki, f32)
    P1_co = np.asarray(Ecorr, f32) @ np.asarray(Wpka1[D:2 * D], f32) + np.asarray(bpka1, f32)
    P2_co = 2.0 * (np.asarray(Ecorr, f32) @ np.asarray(Wpka2[D:2 * D], f32) + np.asarray(bpka2, f32))
    ki = (KI_qd[:, None, None, :] + KI_cd[None, :, None, :] + KI_co[None, None, :, :])
    ki = ki.reshape(NQD * NCD * 2, D)
    p1 = np.broadcast_to(P1_co[None, None, :, :], (NQD, NCD, 2, D)).reshape(-1, D)
    p2 = np.broadcast_to(P2_co[None, None, :, :], (NQD, NCD, 2, D)).reshape(-1, D)
    COMB = np.concatenate([ki, p1, p2], axis=1).astype(f32)   # [20000, 384]
    return dict(
        T_q=np.ascontiguousarray(T_q, f32),
        T_c=np.ascontiguousarray(T_c, f32),
        T_qdcd=np.ascontiguousarray(T_qdcd, f32),
        COMB=np.ascontiguousarray(COMB, f32),
        Wsdf1p=np.ascontiguousarray(Wsdf1, f32),          # +Wsdf1 (x side)
        Wsdf2p2=np.ascontiguousarray(2.0 * Wsdf2, f32),   # +2*Wsdf2 (x side)
        W1n=np.ascontiguousarray(-np.asarray(Wsdf1, f32)),
        W2n2=np.ascontiguousarray(-2.0 * np.asarray(Wsdf2, f32)),
        Wk1=np.ascontiguousarray(Wki[0:D], f32),
        Wp1=np.ascontiguousarray(Wpka1[0:D], f32),
        Wp1N=np.ascontiguousarray(-np.asarray(Wpka1[0:D], f32)),
        Wp2x2=np.ascontiguousarray(2.0 * np.asarray(Wpka2[0:D], f32)),
        Wp2N2=np.ascontiguousarray(-2.0 * np.asarray(Wpka2[0:D], f32)),
    )


def _group_idx(arr_sb):   # [nsteps, BC] step-major -> [128, NGRP] int32 (pad 0)
    flat = arr_sb.reshape(-1)
    pad = NGRP * GTOK - flat.shape[0]
    if pad:
        flat = np.concatenate([flat, np.zeros(pad, flat.dtype)])
    return np.ascontiguousarray(flat.reshape(NGRP, GTOK).T.astype(np.int32))


def _build_program():
    import concourse.bacc as bacc
    import concourse.bass as bass
    import concourse.mybir as mybir
    from concourse.tile import TileContext
    from concourse.masks import make_identity

    f32 = mybir.dt.float32
    Alu = mybir.AluOpType
    Act = mybir.ActivationFunctionType
    nc = bacc.Bacc("TRN2", target_bir_lowering=False, debug=False,
                   num_devices=NCORES, num_swdge_queues=4)

    dram = {}
    for nm, shape, dt in [
        ("T_q", (NQ, D), f32), ("T_c", (NC, D), f32), ("T_qdcd", (NQD * NCD, D), f32),
        ("COMB", (NQD * NCD * 2, 3 * D), f32),
        ("Wsdf1p", (D, D), f32), ("Wsdf2p2", (D, D), f32), ("W1n", (D, D), f32),
        ("W2n2", (D, D), f32), ("Wk1", (D, D), f32), ("Wp1", (D, D), f32),
        ("Wp1N", (D, D), f32), ("Wp2x2", (D, D), f32), ("Wp2N2", (D, D), f32),
        ("h0T", (D, BC), f32),
        ("qidx", (128, NGRP), mybir.dt.int32), ("cidx", (128, NGRP), mybir.dt.int32),
        ("qdcdidx", (128, NGRP), mybir.dt.int32), ("combidx", (128, NGRP), mybir.dt.int32),
    ]:
        dram[nm] = nc.dram_tensor(nm, shape, dt, kind="ExternalInput")
    t_y = nc.dram_tensor("y", (NCHUNK * CH * BC,), f32, kind="ExternalOutput")

    def gather(out_ap, table, idx_col, queue, accum=False):
        inst = nc.gpsimd.indirect_dma_start(
            out=out_ap, out_offset=None, in_=dram[table].ap(),
            in_offset=bass.IndirectOffsetOnAxis(ap=idx_col, axis=0),
            compute_op=Alu.add if accum else Alu.bypass,
        )
        inst.ins.queue = f"qPoolDynamic{queue or ''}"
        return inst

    with TileContext(nc) as tc:
        with (
            tc.tile_pool(name="const", bufs=1) as cpool,
            tc.tile_pool(name="gath", bufs=3) as gpool,
            tc.tile_pool(name="xt", bufs=3) as xtpool,
            tc.tile_pool(name="step", bufs=3) as spool,
            tc.tile_pool(name="hpool", bufs=3) as hpool,
            tc.tile_pool(name="ppsum", bufs=2, space="PSUM") as ppool,
            tc.tile_pool(name="xpsum", bufs=2, space="PSUM") as xppool,
        ):
            ident = cpool.tile([128, 128], f32)
            make_identity(nc, ident)
            ones_col = cpool.tile([128, 1], f32)
            nc.vector.memset(ones_col[:], 1.0)
            w_sb = {}
            for nm in ["Wsdf1p", "Wsdf2p2", "W1n", "W2n2", "Wk1", "Wp1", "Wp1N", "Wp2x2", "Wp2N2"]:
                w_sb[nm] = cpool.tile([D, D], f32, name=nm, tag=nm)
                nc.sync.dma_start(out=w_sb[nm][:], in_=dram[nm].ap())
            idx_sb = {}
            for nm in ["qidx", "cidx", "qdcdidx", "combidx"]:
                idx_sb[nm] = cpool.tile([128, NGRP], mybir.dt.int32, name=nm, tag=nm)
                nc.sync.dma_start(out=idx_sb[nm][:], in_=dram[nm].ap())
            h = hpool.tile([D, BC], f32, tag="h")
            nc.sync.dma_start(out=h[:], in_=dram["h0T"].ap())

            # deferred y state: (prod_tile, cp_base_ap, h_at_boundary, nst_prev, k_prev)
            pending = None

            for k in range(NCHUNK):
                nst = min(CH, NSTEP - k * CH)
                g0 = 2 * k
                # ---- gathers (token-major rows); one tile per group so each
                # consumer waits on exactly one DMA-queue proc ----
                xgs, cgs = [], []
                for g in range(2):
                    xg = gpool.tile([128, D], f32, tag=f"xg{g}")
                    gather(xg[:], "T_q", idx_sb["qidx"][:, g0 + g:g0 + g + 1], g % 2)
                    gather(xg[:], "T_c", idx_sb["cidx"][:, g0 + g:g0 + g + 1], g % 2, accum=True)
                    gather(xg[:], "T_qdcd", idx_sb["qdcdidx"][:, g0 + g:g0 + g + 1], g % 2, accum=True)
                    xgs.append(xg)
                    cg = gpool.tile([128, 3 * D], f32, tag=f"cg{g}")
                    gather(cg[:], "COMB", idx_sb["combidx"][:, g0 + g:g0 + g + 1], 2 + (g % 2))
                    cgs.append(cg)

                # ---- x^T via PE transpose -> psum -> sbuf ----
                xps = xppool.tile([128, 2 * D], f32, tag="xps")
                for g in range(2):
                    nc.tensor.transpose(out=xps[:, g * D:(g + 1) * D],
                                        in_=xgs[g][:], identity=ident[:])
                xT = xtpool.tile([128, 2 * D], f32, tag="xT")
                nc.vector.tensor_copy(xT[:], xps[:])

                # ---- flush previous chunk's boundary prod + y ----
                if pending is not None:
                    pprod, pct, pco, pca, ph, pnst, pk = pending
                    nc.gpsimd.tensor_tensor(out=pprod[:, (CH - 1) * 64:CH * 64],
                                            in0=ph[:], in1=xT[:, 0:64], op=Alu.mult)
                    nc.tensor.matmul(bass.AP(pct, pco + 1280, [[pca[0][0], 1], [1, 64 * pnst]]),
                                     ones_col[:], pprod[:, 0:64 * pnst],
                                     start=False, stop=True, skip_group_check=True)
                    ysb = spool.tile([1, 256], f32, tag="ysb")
                    nc.scalar.activation(ysb[:1, 0:64 * pnst],
                                         bass.AP(pct, pco + 1280, [[pca[0][0], 1], [1, 64 * pnst]]),
                                         Act.Sigmoid)
                    nc.sync.dma_start(out=t_y.ap()[pk * CH * BC: pk * CH * BC + 64 * pnst],
                                      in_=ysb[:1, 0:64 * pnst])
                    pending = None

                # ---- chunk psum: bankA = sdf1|sdf2', bankB = ki|pka1, bankC = pka2'|y ----
                cp = ppool.tile([128, 3 * 512], f32, tag="cp")
                base = cp[:]
                ct, co, ca = base.tensor, base.offset, base.ap

                def cps(col0, ncols):
                    return bass.AP(ct, co + col0, [[ca[0][0], 128], [1, ncols]])

                for g in range(2):   # ki bases -> bankB cols 0..255 (abs 512..767)
                    nc.tensor.matmul(cps(512 + g * 128, 128),
                                     cgs[g][:, 0:D], ident[:],
                                     start=(g == 0), stop=False,
                                     is_transpose=True, skip_group_check=True)
                for g in range(2):   # pka1 bases -> bankB cols 256..511
                    nc.tensor.matmul(cps(768 + g * 128, 128),
                                     cgs[g][:, D:2 * D], ident[:],
                                     start=False, stop=False,
                                     is_transpose=True, skip_group_check=True)
                for g in range(2):   # pka2' bases -> bankC cols 0..255
                    nc.tensor.matmul(cps(1024 + g * 128, 128),
                                     cgs[g][:, 2 * D:3 * D], ident[:],
                                     start=(g == 0), stop=False,
                                     is_transpose=True, skip_group_check=True)
                # x side of sdf gates -> bankA
                nc.tensor.matmul(cps(0, 256), w_sb["Wsdf1p"][:], xT[:],
                                 start=True, stop=False, skip_group_check=True)
                nc.tensor.matmul(cps(256, 256), w_sb["Wsdf2p2"][:], xT[:],
                                 start=False, stop=False, skip_group_check=True)

                prod = spool.tile([128, 256], f32, tag="prod")

                for s in range(nst):
                    nc.tensor.matmul(cps(0 + s * 64, 64), w_sb["W1n"][:], h[:],
                                     start=False, stop=False, skip_group_check=True)
                    nc.tensor.matmul(cps(256 + s * 64, 64), w_sb["W2n2"][:], h[:],
                                     start=False, stop=False, skip_group_check=True)
                    nc.tensor.matmul(cps(512 + s * 64, 64), w_sb["Wk1"][:], h[:],
                                     start=False, stop=False, skip_group_check=True)
                    gates1 = spool.tile([128, 192], f32, tag="gates1")
                    a1src = bass.AP(ct, co + s * 64, [[ca[0][0], 128], [256, 3], [1, 64]])
                    a1dst = gates1[:].rearrange("p (a b) -> p a b", b=64)
                    nc.scalar.activation(a1dst, a1src, Act.Sigmoid)
                    s1, s2p, gam = gates1[:, 0:64], gates1[:, 64:128], gates1[:, 128:192]
                    m = spool.tile([128, 64], f32, tag="m")
                    nc.vector.scalar_tensor_tensor(out=m[:], in0=s2p, scalar=2.0, in1=s1,
                                                   op0=Alu.mult, op1=Alu.mult)
                    # Wp@(m - s1) split as Wp@m + (-Wp)@s1: kills the sdf
                    # DVE hop on the critical path (s1 ready at act1 already)
                    nc.tensor.matmul(cps(768 + s * 64, 64), w_sb["Wp1N"][:], s1,
                                     start=False, stop=False, skip_group_check=True)
                    nc.tensor.matmul(cps(1024 + s * 64, 64), w_sb["Wp2N2"][:], s1,
                                     start=False, stop=False, skip_group_check=True)
                    nc.tensor.matmul(cps(768 + s * 64, 64), w_sb["Wp1"][:], m[:],
                                     start=False, stop=False, skip_group_check=True)
                    nc.tensor.matmul(cps(1024 + s * 64, 64), w_sb["Wp2x2"][:], m[:],
                                     start=False, stop=False, skip_group_check=True)
                    gates2 = spool.tile([128, 128], f32, tag="gates2")
                    a2src = bass.AP(ct, co + 768 + s * 64, [[ca[0][0], 128], [256, 2], [1, 64]])
                    a2dst = gates2[:].rearrange("p (a b) -> p a b", b=64)
                    nc.scalar.activation(a2dst, a2src, Act.Sigmoid)
                    p1, p2p = gates2[:, 0:64], gates2[:, 64:128]
                    m2 = spool.tile([128, 64], f32, tag="m2")
                    nc.vector.scalar_tensor_tensor(out=m2[:], in0=p2p, scalar=2.0, in1=p1,
                                                   op0=Alu.mult, op1=Alu.mult)
                    pka = spool.tile([128, 64], f32, tag="pka")
                    nc.vector.tensor_tensor(out=pka[:], in0=m2[:], in1=p1, op=Alu.subtract)
                    # h' = gam*h + (1-gam)*pka
                    gamc = spool.tile([128, 64], f32, tag="gamc")
                    nc.gpsimd.tensor_scalar(out=gamc[:], in0=gam, scalar1=-1.0, scalar2=1.0,
                                            op0=Alu.mult, op1=Alu.add)
                    g1 = spool.tile([128, 64], f32, tag="g1")
                    nc.vector.tensor_tensor(out=g1[:], in0=gam, in1=h[:], op=Alu.mult)
                    u = spool.tile([128, 64], f32, tag="u")
                    nc.gpsimd.tensor_tensor(out=u[:], in0=gamc[:], in1=pka[:], op=Alu.mult)
                    hn = hpool.tile([D, BC], f32, tag="h")
                    nc.vector.tensor_tensor(out=hn[:], in0=g1[:], in1=u[:], op=Alu.add)
                    h = hn
                    if s < nst - 1 or k == NCHUNK - 1:
                        nc.gpsimd.tensor_tensor(out=prod[:, s * 64:(s + 1) * 64],
                                                in0=h[:], in1=xT[:, (s + 1) * 64:(s + 2) * 64],
                                                op=Alu.mult)

                if k == NCHUNK - 1:
                    nc.tensor.matmul(bass.AP(ct, co + 1280, [[ca[0][0], 1], [1, 64 * nst]]),
                                     ones_col[:], prod[:, 0:64 * nst],
                                     start=False, stop=True, skip_group_check=True)
                    ysb = spool.tile([1, 256], f32, tag="ysb")
                    nc.scalar.activation(ysb[:1, 0:64 * nst],
                                         bass.AP(ct, co + 1280, [[ca[0][0], 1], [1, 64 * nst]]),
                                         Act.Sigmoid)
                    nc.sync.dma_start(out=t_y.ap()[k * CH * BC: k * CH * BC + 64 * nst],
                                      in_=ysb[:1, 0:64 * nst])
                else:
                    pending = (prod, ct, co, ca, h, nst, k)
    nc.compile()
    return nc


def kernel(**inputs):
    from concourse.bass_utils import run_bass_kernel_spmd

    w = _host_pack(**{k: np.asarray(inputs[k]) for k in
                      ["Eq", "Ec", "Eqd", "Ecd", "Ecorr", "Wx", "bx", "Wsdf1", "bsdf1",
                       "Wsdf2", "bsdf2", "Wpka1", "bpka1", "Wpka2", "bpka2", "Wki", "bki"]})
    q = np.asarray(inputs["question_seq"])
    c = np.asarray(inputs["concept_seq"])
    qd = np.asarray(inputs["question_diff_seq"])
    cd = np.asarray(inputs["concept_diff_seq"])
    co = np.asarray(inputs["correct_seq"])
    h0 = np.asarray(inputs["h0"], np.float32)
    qdcd = (qd * NCD + cd).astype(np.int64)
    comb = (qd * (NCD * 2) + cd * 2 + co).astype(np.int64)

    if "nc" not in _cache:
        _cache["nc"] = _build_program()
    nc = _cache["nc"]

    in_maps = []
    for core in range(NCORES):
        rows = slice(core * BC, (core + 1) * BC)
        m = dict(w)
        m["h0T"] = np.ascontiguousarray(h0[rows].T)
        m["qidx"] = _group_idx(q[rows].T)          # [S, BC] step-major
        m["cidx"] = _group_idx(c[rows].T)
        m["qdcdidx"] = _group_idx(qdcd[rows].T)
        m["combidx"] = _group_idx(comb[rows].T[:NSTEP])
        in_maps.append(m)

    global _last_in_maps
    _last_in_maps = in_maps
    res = run_bass_kernel_spmd(nc, in_maps, list(range(NCORES)))
    y = np.zeros((B, S), np.float32)
    for core in range(NCORES):
        yd = res.results[core]["y"][:NSTEP * BC].reshape(NSTEP, BC)
        y[core * BC:(core + 1) * BC, :NSTEP] = yd.T
    return y



# revision 9
# speedup vs baseline: 1.1486x; 1.0459x over previous
"""DIMKT scan kernel for 8x Trainium2 NeuronCores (Bass/Tile).

Data-parallel over batch (64 rows/core). Host packs derived weight tables
(weight-side transforms only); device gathers per-token rows, transposes them
into PSUM as gate accumulation bases, and runs the sequential scan with
5 small matmuls + 2 strided sigmoids (tanh(x) = 2*sigmoid(2x) - 1) per step.
y_t = sigmoid(dot(x_{t+1}, h_t)) via a ones-column matmul batched per chunk.
"""
import numpy as np

B, S, D = 512, 500, 128
NQ, NC, NQD, NCD = 10000, 500, 100, 100
NCORES = 8
BC = B // NCORES          # 64 batch rows per core
CH = 4                    # timesteps per chunk
NSTEP = S - 1             # 499 scan steps
NCHUNK = (NSTEP + CH - 1) // CH   # 125 (last chunk has 3 steps)
XTOK = S * BC             # 32000 x tokens per core
GTOK = 128                # tokens per gather group
NGRP = XTOK // GTOK       # 250 groups

_cache = {}


def _host_pack(Eq, Ec, Eqd, Ecd, Ecorr, Wx, bx, Wsdf1, bsdf1, Wsdf2, bsdf2,
               Wpka1, bpka1, Wpka2, bpka2, Wki, bki):
    f32 = np.float32
    Wx0, Wx1, Wx2, Wx3 = (np.asarray(Wx[i * D:(i + 1) * D], f32) for i in range(4))
    T_q = np.asarray(Eq, f32) @ Wx0
    T_c = np.asarray(Ec, f32) @ Wx1 + np.asarray(bx, f32)
    A = np.asarray(Eqd, f32) @ Wx2            # [100,128]
    Bt = np.asarray(Ecd, f32) @ Wx3           # [100,128]
    T_qdcd = (A[:, None, :] + Bt[None, :, :]).reshape(NQD * NCD, D).astype(f32)
    # COMB[(qd*200 + cd*2 + co)] rows: [ki_part | pka1_part | 2*pka2_part]
    KI_qd = np.asarray(Eqd, f32) @ np.asarray(Wki[2 * D:3 * D], f32)
    KI_cd = np.asarray(Ecd, f32) @ np.asarray(Wki[3 * D:4 * D], f32)
    KI_co = np.asarray(Ecorr, f32) @ np.asarray(Wki[D:2 * D], f32) + np.asarray(bki, f32)
    P1_co = np.asarray(Ecorr, f32) @ np.asarray(Wpka1[D:2 * D], f32) + np.asarray(bpka1, f32)
    P2_co = 2.0 * (np.asarray(Ecorr, f32) @ np.asarray(Wpka2[D:2 * D], f32) + np.asarray(bpka2, f32))
    ki = (KI_qd[:, None, None, :] + KI_cd[None, :, None, :] + KI_co[None, None, :, :])
    ki = ki.reshape(NQD * NCD * 2, D)
    p1 = np.broadcast_to(P1_co[None, None, :, :], (NQD, NCD, 2, D)).reshape(-1, D)
    p2 = np.broadcast_to(P2_co[None, None, :, :], (NQD, NCD, 2, D)).reshape(-1, D)
    COMB = np.concatenate([ki, p1, p2], axis=1).astype(f32)   # [20000, 384]
    return dict(
        T_q=np.ascontiguousarray(T_q, f32),
        T_c=np.ascontiguousarray(T_c, f32),
        T_qdcd=np.ascontiguousarray(T_qdcd, f32),
        COMB=np.ascontiguousarray(COMB, f32),
        Wsdf1p=np.ascontiguousarray(Wsdf1, f32),          # +Wsdf1 (x side)
        Wsdf2p2=np.ascontiguousarray(2.0 * Wsdf2, f32),   # +2*Wsdf2 (x side)
        W1n=np.ascontiguousarray(-np.asarray(Wsdf1, f32)),
        W2n2=np.ascontiguousarray(-2.0 * np.asarray(Wsdf2, f32)),
        Wk1=np.ascontiguousarray(Wki[0:D], f32),
        Wp1=np.ascontiguousarray(Wpka1[0:D], f32),
        Wp1N=np.ascontiguousarray(-np.asarray(Wpka1[0:D], f32)),
        Wp2x2=np.ascontiguousarray(2.0 * np.asarray(Wpka2[0:D], f32)),
        Wp2N2=np.ascontiguousarray(-2.0 * np.asarray(Wpka2[0:D], f32)),
    )


def _group_idx(arr_sb):   # [nsteps, BC] step-major -> [128, NGRP] int32 (pad 0)
    flat = arr_sb.reshape(-1)
    pad = NGRP * GTOK - flat.shape[0]
    if pad:
        flat = np.concatenate([flat, np.zeros(pad, flat.dtype)])
    return np.ascontiguousarray(flat.reshape(NGRP, GTOK).T.astype(np.int32))


def _build_program():
    import concourse.bacc as bacc
    import concourse.bass as bass
    import concourse.mybir as mybir
    from concourse.tile import TileContext
    from concourse.masks import make_identity

    f32 = mybir.dt.float32
    Alu = mybir.AluOpType
    Act = mybir.ActivationFunctionType
    nc = bacc.Bacc("TRN2", target_bir_lowering=False, debug=False,
                   num_devices=NCORES, num_swdge_queues=4)

    dram = {}
    for nm, shape, dt in [
        ("T_q", (NQ, D), f32), ("T_c", (NC, D), f32), ("T_qdcd", (NQD * NCD, D), f32),
        ("COMB", (NQD * NCD * 2, 3 * D), f32),
        ("Wsdf1p", (D, D), f32), ("Wsdf2p2", (D, D), f32), ("W1n", (D, D), f32),
        ("W2n2", (D, D), f32), ("Wk1", (D, D), f32), ("Wp1", (D, D), f32),
        ("Wp1N", (D, D), f32), ("Wp2x2", (D, D), f32), ("Wp2N2", (D, D), f32),
        ("h0T", (D, BC), f32),
        ("qidx", (128, NGRP), mybir.dt.int32), ("cidx", (128, NGRP), mybir.dt.int32),
        ("qdcdidx", (128, NGRP), mybir.dt.int32), ("combidx", (128, NGRP), mybir.dt.int32),
    ]:
        dram[nm] = nc.dram_tensor(nm, shape, dt, kind="ExternalInput")
    t_y = nc.dram_tensor("y", (NCHUNK * CH * BC,), f32, kind="ExternalOutput")

    def gather(out_ap, table, idx_col, queue, accum=False):
        inst = nc.gpsimd.indirect_dma_start(
            out=out_ap, out_offset=None, in_=dram[table].ap(),
            in_offset=bass.IndirectOffsetOnAxis(ap=idx_col, axis=0),
            compute_op=Alu.add if accum else Alu.bypass,
        )
        inst.ins.queue = f"qPoolDynamic{queue or ''}"
        return inst

    with TileContext(nc) as tc:
        with (
            tc.tile_pool(name="const", bufs=1) as cpool,
            tc.tile_pool(name="gath", bufs=3) as gpool,
            tc.tile_pool(name="xt", bufs=3) as xtpool,
            tc.tile_pool(name="step", bufs=3) as spool,
            tc.tile_pool(name="hpool", bufs=3) as hpool,
            tc.tile_pool(name="ppsum", bufs=2, space="PSUM") as ppool,
            tc.tile_pool(name="xpsum", bufs=2, space="PSUM") as xppool,
        ):
            ident = cpool.tile([128, 128], f32)
            make_identity(nc, ident)
            ones_col = cpool.tile([128, 1], f32)
            nc.vector.memset(ones_col[:], 1.0)
            w_sb = {}
            for nm in ["Wsdf1p", "Wsdf2p2", "W1n", "W2n2", "Wk1", "Wp1", "Wp1N", "Wp2x2", "Wp2N2"]:
                w_sb[nm] = cpool.tile([D, D], f32, name=nm, tag=nm)
                nc.sync.dma_start(out=w_sb[nm][:], in_=dram[nm].ap())
            idx_sb = {}
            for nm in ["qidx", "cidx", "qdcdidx", "combidx"]:
                idx_sb[nm] = cpool.tile([128, NGRP], mybir.dt.int32, name=nm, tag=nm)
                nc.sync.dma_start(out=idx_sb[nm][:], in_=dram[nm].ap())
            h = hpool.tile([D, BC], f32, tag="h")
            nc.sync.dma_start(out=h[:], in_=dram["h0T"].ap())

            # deferred y state: (prod_tile, cp_base_ap, h_at_boundary, nst_prev, k_prev)
            pending = None

            for k in range(NCHUNK):
                nst = min(CH, NSTEP - k * CH)
                g0 = 2 * k
                # ---- gathers (token-major rows); one tile per group so each
                # consumer waits on exactly one DMA-queue proc ----
                xgs, cgs = [], []
                for g in range(2):
                    xg = gpool.tile([128, D], f32, tag=f"xg{g}")
                    gather(xg[:], "T_q", idx_sb["qidx"][:, g0 + g:g0 + g + 1], g % 2)
                    gather(xg[:], "T_c", idx_sb["cidx"][:, g0 + g:g0 + g + 1], g % 2, accum=True)
                    gather(xg[:], "T_qdcd", idx_sb["qdcdidx"][:, g0 + g:g0 + g + 1], g % 2, accum=True)
                    xgs.append(xg)
                    cg = gpool.tile([128, 3 * D], f32, tag=f"cg{g}")
                    gather(cg[:], "COMB", idx_sb["combidx"][:, g0 + g:g0 + g + 1], 2 + (g % 2))
                    cgs.append(cg)

                # ---- x^T via PE transpose -> psum -> sbuf ----
                xps = xppool.tile([128, 2 * D], f32, tag="xps")
                for g in range(2):
                    nc.tensor.transpose(out=xps[:, g * D:(g + 1) * D],
                                        in_=xgs[g][:], identity=ident[:])
                xT = xtpool.tile([128, 2 * D], f32, tag="xT")
                nc.vector.tensor_copy(xT[:], xps[:])

                # ---- flush previous chunk's boundary prod + y ----
                if pending is not None:
                    pprod, pct, pco, pca, ph, pnst, pk = pending
                    nc.gpsimd.tensor_tensor(out=pprod[:, (CH - 1) * 64:CH * 64],
                                            in0=ph[:], in1=xT[:, 0:64], op=Alu.mult)
                    nc.tensor.matmul(bass.AP(pct, pco + 1280, [[pca[0][0], 1], [1, 64 * pnst]]),
                                     ones_col[:], pprod[:, 0:64 * pnst],
                                     start=False, stop=True, skip_group_check=True)
                    ysb = spool.tile([1, 256], f32, tag="ysb")
                    nc.scalar.activation(ysb[:1, 0:64 * pnst],
                                         bass.AP(pct, pco + 1280, [[pca[0][0], 1], [1, 64 * pnst]]),
                                         Act.Sigmoid)
                    nc.sync.dma_start(out=t_y.ap()[pk * CH * BC: pk * CH * BC + 64 * pnst],
                                      in_=ysb[:1, 0:64 * pnst])
                    pending = None

                # ---- chunk psum: bankA = sdf1|sdf2', bankB = ki|pka1, bankC = pka2'|y ----
                cp = ppool.tile([128, 3 * 512], f32, tag="cp")
                base = cp[:]
                ct, co, ca = base.tensor, base.offset, base.ap

                def cps(col0, ncols):
                    return bass.AP(ct, co + col0, [[ca[0][0], 128], [1, ncols]])

                for g in range(2):   # ki bases -> bankB cols 0..255 (abs 512..767)
                    nc.tensor.matmul(cps(512 + g * 128, 128),
                                     cgs[g][:, 0:D], ident[:],
                                     start=(g == 0), stop=False,
                                     is_transpose=True, skip_group_check=True)
                for g in range(2):   # pka1 bases -> bankB cols 256..511
                    nc.tensor.matmul(cps(768 + g * 128, 128),
                                     cgs[g][:, D:2 * D], ident[:],
                                     start=False, stop=False,
                                     is_transpose=True, skip_group_check=True)
                for g in range(2):   # pka2' bases -> bankC cols 0..255
                    nc.tensor.matmul(cps(1024 + g * 128, 128),
                                     cgs[g][:, 2 * D:3 * D], ident[:],
                                     start=(g == 0), stop=False,
                                     is_transpose=True, skip_group_check=True)
                # x side of sdf gates -> bankA
                nc.tensor.matmul(cps(0, 256), w_sb["Wsdf1p"][:], xT[:],
                                 start=True, stop=False, skip_group_check=True)
                nc.tensor.matmul(cps(256, 256), w_sb["Wsdf2p2"][:], xT[:],
                                 start=False, stop=False, skip_group_check=True)

                prod = spool.tile([128, 256], f32, tag="prod")

                for s in range(nst):
                    nc.tensor.matmul(cps(0 + s * 64, 64), w_sb["W1n"][:], h[:],
                                     start=False, stop=False, skip_group_check=True)
                    nc.tensor.matmul(cps(256 + s * 64, 64), w_sb["W2n2"][:], h[:],
                                     start=False, stop=False, skip_group_check=True)
                    nc.tensor.matmul(cps(512 + s * 64, 64), w_sb["Wk1"][:], h[:],
                                     start=False, stop=False, skip_group_check=True)
                    gates1 = spool.tile([128, 192], f32, tag="gates1")
                    a1src = bass.AP(ct, co + s * 64, [[ca[0][0], 128], [256, 3], [1, 64]])
                    a1dst = gates1[:].rearrange("p (a b) -> p a b", b=64)
                    nc.scalar.activation(a1dst, a1src, Act.Sigmoid)
                    s1, s2p, gam = gates1[:, 0:64], gates1[:, 64:128], gates1[:, 128:192]
                    m = spool.tile([128, 64], f32, tag="m")
                    nc.vector.scalar_tensor_tensor(out=m[:], in0=s2p, scalar=2.0, in1=s1,
                                                   op0=Alu.mult, op1=Alu.mult)
                    # Wp@(m - s1) split as Wp@m + (-Wp)@s1: kills the sdf
                    # DVE hop on the critical path (s1 ready at act1 already)
                    nc.tensor.matmul(cps(768 + s * 64, 64), w_sb["Wp1N"][:], s1,
                                     start=False, stop=False, skip_group_check=True)
                    nc.tensor.matmul(cps(1024 + s * 64, 64), w_sb["Wp2N2"][:], s1,
                                     start=False, stop=False, skip_group_check=True)
                    nc.tensor.matmul(cps(768 + s * 64, 64), w_sb["Wp1"][:], m[:],
                                     start=False, stop=False, skip_group_check=True)
                    nc.tensor.matmul(cps(1024 + s * 64, 64), w_sb["Wp2x2"][:], m[:],
                                     start=False, stop=False, skip_group_check=True)
                    gates2 = spool.tile([128, 128], f32, tag="gates2")
                    a2src = bass.AP(ct, co + 768 + s * 64, [[ca[0][0], 128], [256, 2], [1, 64]])
                    a2dst = gates2[:].rearrange("p (a b) -> p a b", b=64)
                    nc.scalar.activation(a2dst, a2src, Act.Sigmoid)
                    p1, p2p = gates2[:, 0:64], gates2[:, 64:128]
                    m2 = spool.tile([128, 64], f32, tag="m2")
                    nc.vector.scalar_tensor_tensor(out=m2[:], in0=p2p, scalar=2.0, in1=p1,
                                                   op0=Alu.mult, op1=Alu.mult)
                    pka = spool.tile([128, 64], f32, tag="pka")
                    nc.vector.tensor_tensor(out=pka[:], in0=m2[:], in1=p1, op=Alu.subtract)
                    # h' = gam*h + (1-gam)*pka
                    gamc = spool.tile([128, 64], f32, tag="gamc")
                    nc.gpsimd.tensor_scalar(out=gamc[:], in0=gam, scalar1=-1.0, scalar2=1.0,
                                            op0=Alu.mult, op1=Alu.add)
                    g1 = spool.tile([128, 64], f32, tag="g1")
                    nc.vector.tensor_tensor(out=g1[:], in0=gam, in1=h[:], op=Alu.mult)
                    u = spool.tile([128, 64], f32, tag="u")
                    nc.vector.tensor_tensor(out=u[:], in0=gamc[:], in1=pka[:], op=Alu.mult)
                    hn = hpool.tile([D, BC], f32, tag="h")
                    nc.vector.tensor_tensor(out=hn[:], in0=g1[:], in1=u[:], op=Alu.add)
                    h = hn
                    if s < nst - 1 or k == NCHUNK - 1:
                        nc.gpsimd.tensor_tensor(out=prod[:, s * 64:(s + 1) * 64],
                                                in0=h[:], in1=xT[:, (s + 1) * 64:(s + 2) * 64],
                                                op=Alu.mult)

                if k == NCHUNK - 1:
                    nc.tensor.matmul(bass.AP(ct, co + 1280, [[ca[0][0], 1], [1, 64 * nst]]),
                                     ones_col[:], prod[:, 0:64 * nst],
                                     start=False, stop=True, skip_group_check=True)
                    ysb = spool.tile([1, 256], f32, tag="ysb")
                    nc.scalar.activation(ysb[:1, 0:64 * nst],
                                         bass.AP(ct, co + 1280, [[ca[0][0], 1], [1, 64 * nst]]),
                                         Act.Sigmoid)
                    nc.sync.dma_start(out=t_y.ap()[k * CH * BC: k * CH * BC + 64 * nst],
                                      in_=ysb[:1, 0:64 * nst])
                else:
                    pending = (prod, ct, co, ca, h, nst, k)
    nc.compile()
    return nc


def kernel(**inputs):
    from concourse.bass_utils import run_bass_kernel_spmd

    w = _host_pack(**{k: np.asarray(inputs[k]) for k in
                      ["Eq", "Ec", "Eqd", "Ecd", "Ecorr", "Wx", "bx", "Wsdf1", "bsdf1",
                       "Wsdf2", "bsdf2", "Wpka1", "bpka1", "Wpka2", "bpka2", "Wki", "bki"]})
    q = np.asarray(inputs["question_seq"])
    c = np.asarray(inputs["concept_seq"])
    qd = np.asarray(inputs["question_diff_seq"])
    cd = np.asarray(inputs["concept_diff_seq"])
    co = np.asarray(inputs["correct_seq"])
    h0 = np.asarray(inputs["h0"], np.float32)
    qdcd = (qd * NCD + cd).astype(np.int64)
    comb = (qd * (NCD * 2) + cd * 2 + co).astype(np.int64)

    if "nc" not in _cache:
        _cache["nc"] = _build_program()
    nc = _cache["nc"]

    in_maps = []
    for core in range(NCORES):
        rows = slice(core * BC, (core + 1) * BC)
        m = dict(w)
        m["h0T"] = np.ascontiguousarray(h0[rows].T)
        m["qidx"] = _group_idx(q[rows].T)          # [S, BC] step-major
        m["cidx"] = _group_idx(c[rows].T)
        m["qdcdidx"] = _group_idx(qdcd[rows].T)
        m["combidx"] = _group_idx(comb[rows].T[:NSTEP])
        in_maps.append(m)

    global _last_in_maps
    _last_in_maps = in_maps
    res = run_bass_kernel_spmd(nc, in_maps, list(range(NCORES)))
    y = np.zeros((B, S), np.float32)
    for core in range(NCORES):
        yd = res.results[core]["y"][:NSTEP * BC].reshape(NSTEP, BC)
        y[core * BC:(core + 1) * BC, :NSTEP] = yd.T
    return y



# revision 10
# speedup vs baseline: 1.2375x; 1.0774x over previous
"""DIMKT scan kernel for 8x Trainium2 NeuronCores (Bass/Tile).

Data-parallel over batch (64 rows/core). Host packs derived weight tables
(weight-side transforms only); device gathers per-token rows, transposes them
into PSUM as gate accumulation bases, and runs the sequential scan with
5 small matmuls + 2 strided sigmoids (tanh(x) = 2*sigmoid(2x) - 1) per step.
y_t = sigmoid(dot(x_{t+1}, h_t)) via a ones-column matmul batched per chunk.
"""
import numpy as np

B, S, D = 512, 500, 128
NQ, NC, NQD, NCD = 10000, 500, 100, 100
NCORES = 8
BC = B // NCORES          # 64 batch rows per core
CH = 4                    # timesteps per chunk
NSTEP = S - 1             # 499 scan steps
NCHUNK = (NSTEP + CH - 1) // CH   # 125 (last chunk has 3 steps)
XTOK = S * BC             # 32000 x tokens per core
GTOK = 128                # tokens per gather group
NGRP = XTOK // GTOK       # 250 groups

_cache = {}


def _host_pack(Eq, Ec, Eqd, Ecd, Ecorr, Wx, bx, Wsdf1, bsdf1, Wsdf2, bsdf2,
               Wpka1, bpka1, Wpka2, bpka2, Wki, bki):
    f32 = np.float32
    Wx0, Wx1, Wx2, Wx3 = (np.asarray(Wx[i * D:(i + 1) * D], f32) for i in range(4))
    T_q = np.asarray(Eq, f32) @ Wx0
    T_c = np.asarray(Ec, f32) @ Wx1 + np.asarray(bx, f32)
    A = np.asarray(Eqd, f32) @ Wx2            # [100,128]
    Bt = np.asarray(Ecd, f32) @ Wx3           # [100,128]
    T_qdcd = (A[:, None, :] + Bt[None, :, :]).reshape(NQD * NCD, D).astype(f32)
    # COMB[(qd*200 + cd*2 + co)] rows: [ki_part | pka1_part | 2*pka2_part]
    KI_qd = np.asarray(Eqd, f32) @ np.asarray(Wki[2 * D:3 * D], f32)
    KI_cd = np.asarray(Ecd, f32) @ np.asarray(Wki[3 * D:4 * D], f32)
    KI_co = np.asarray(Ecorr, f32) @ np.asarray(Wki[D:2 * D], f32) + np.asarray(bki, f32)
    P1_co = np.asarray(Ecorr, f32) @ np.asarray(Wpka1[D:2 * D], f32) + np.asarray(bpka1, f32)
    P2_co = 2.0 * (np.asarray(Ecorr, f32) @ np.asarray(Wpka2[D:2 * D], f32) + np.asarray(bpka2, f32))
    ki = (KI_qd[:, None, None, :] + KI_cd[None, :, None, :] + KI_co[None, None, :, :])
    ki = ki.reshape(NQD * NCD * 2, D)
    p1 = np.broadcast_to(P1_co[None, None, :, :], (NQD, NCD, 2, D)).reshape(-1, D)
    p2 = np.broadcast_to(P2_co[None, None, :, :], (NQD, NCD, 2, D)).reshape(-1, D)
    COMB = np.concatenate([ki, p1, p2], axis=1).astype(f32)   # [20000, 384]
    return dict(
        T_q=np.ascontiguousarray(T_q, f32),
        T_c=np.ascontiguousarray(T_c, f32),
        T_qdcd=np.ascontiguousarray(T_qdcd, f32),
        COMB=np.ascontiguousarray(COMB, f32),
        Wsdf1p=np.ascontiguousarray(Wsdf1, f32),          # +Wsdf1 (x side)
        Wsdf2p2=np.ascontiguousarray(2.0 * Wsdf2, f32),   # +2*Wsdf2 (x side)
        W1n=np.ascontiguousarray(-np.asarray(Wsdf1, f32)),
        W2n2=np.ascontiguousarray(-2.0 * np.asarray(Wsdf2, f32)),
        Wk1=np.ascontiguousarray(Wki[0:D], f32),
        Wp1=np.ascontiguousarray(Wpka1[0:D], f32),
        Wp1N=np.ascontiguousarray(-np.asarray(Wpka1[0:D], f32)),
        Wp2x2=np.ascontiguousarray(2.0 * np.asarray(Wpka2[0:D], f32)),
        Wp2N2=np.ascontiguousarray(-2.0 * np.asarray(Wpka2[0:D], f32)),
    )


def _group_idx(arr_sb):   # [nsteps, BC] step-major -> [128, NGRP] int32 (pad 0)
    flat = arr_sb.reshape(-1)
    pad = NGRP * GTOK - flat.shape[0]
    if pad:
        flat = np.concatenate([flat, np.zeros(pad, flat.dtype)])
    return np.ascontiguousarray(flat.reshape(NGRP, GTOK).T.astype(np.int32))


def _build_program():
    import concourse.bacc as bacc
    import concourse.bass as bass
    import concourse.mybir as mybir
    from concourse.tile import TileContext
    from concourse.masks import make_identity

    f32 = mybir.dt.float32
    Alu = mybir.AluOpType
    Act = mybir.ActivationFunctionType
    nc = bacc.Bacc("TRN2", target_bir_lowering=False, debug=False,
                   num_devices=NCORES, num_swdge_queues=4)

    dram = {}
    for nm, shape, dt in [
        ("T_q", (NQ, D), f32), ("T_c", (NC, D), f32), ("T_qdcd", (NQD * NCD, D), f32),
        ("COMB", (NQD * NCD * 2, 3 * D), f32),
        ("Wsdf1p", (D, D), f32), ("Wsdf2p2", (D, D), f32), ("W1n", (D, D), f32),
        ("W2n2", (D, D), f32), ("Wk1", (D, D), f32), ("Wp1", (D, D), f32),
        ("Wp1N", (D, D), f32), ("Wp2x2", (D, D), f32), ("Wp2N2", (D, D), f32),
        ("h0T", (D, BC), f32),
        ("qidx", (128, NGRP), mybir.dt.int32), ("cidx", (128, NGRP), mybir.dt.int32),
        ("qdcdidx", (128, NGRP), mybir.dt.int32), ("combidx", (128, NGRP), mybir.dt.int32),
    ]:
        dram[nm] = nc.dram_tensor(nm, shape, dt, kind="ExternalInput")
    t_y = nc.dram_tensor("y", (NCHUNK * CH * BC,), f32, kind="ExternalOutput")

    def gather(out_ap, table, idx_col, queue, accum=False):
        inst = nc.gpsimd.indirect_dma_start(
            out=out_ap, out_offset=None, in_=dram[table].ap(),
            in_offset=bass.IndirectOffsetOnAxis(ap=idx_col, axis=0),
            compute_op=Alu.add if accum else Alu.bypass,
        )
        inst.ins.queue = f"qPoolDynamic{queue or ''}"
        return inst

    with TileContext(nc) as tc:
        with (
            tc.tile_pool(name="const", bufs=1) as cpool,
            tc.tile_pool(name="gath", bufs=3) as gpool,
            tc.tile_pool(name="xt", bufs=3) as xtpool,
            tc.tile_pool(name="step", bufs=3) as spool,
            tc.tile_pool(name="hpool", bufs=3) as hpool,
            tc.tile_pool(name="ppsum", bufs=2, space="PSUM") as ppool,
            tc.tile_pool(name="xpsum", bufs=2, space="PSUM") as xppool,
        ):
            ident = cpool.tile([128, 128], f32)
            make_identity(nc, ident)
            ones_col = cpool.tile([128, 1], f32)
            nc.vector.memset(ones_col[:], 1.0)
            w_sb = {}
            for nm in ["Wsdf1p", "Wsdf2p2", "W1n", "W2n2", "Wk1", "Wp1", "Wp1N", "Wp2x2", "Wp2N2"]:
                w_sb[nm] = cpool.tile([D, D], f32, name=nm, tag=nm)
                nc.sync.dma_start(out=w_sb[nm][:], in_=dram[nm].ap())
            idx_sb = {}
            for nm in ["qidx", "cidx", "qdcdidx", "combidx"]:
                idx_sb[nm] = cpool.tile([128, NGRP], mybir.dt.int32, name=nm, tag=nm)
                nc.sync.dma_start(out=idx_sb[nm][:], in_=dram[nm].ap())
            h = hpool.tile([D, BC], f32, tag="h")
            nc.sync.dma_start(out=h[:], in_=dram["h0T"].ap())

            # deferred y state: (prod_tile, cp_base_ap, h_at_boundary, nst_prev, k_prev)
            pending = None

            for k in range(NCHUNK):
                nst = min(CH, NSTEP - k * CH)
                g0 = 2 * k
                # ---- gathers (token-major rows); one tile per group so each
                # consumer waits on exactly one DMA-queue proc ----
                xgs, cgs = [], []
                for g in range(2):
                    xg = gpool.tile([128, D], f32, tag=f"xg{g}")
                    gather(xg[:], "T_q", idx_sb["qidx"][:, g0 + g:g0 + g + 1], g % 2)
                    gather(xg[:], "T_c", idx_sb["cidx"][:, g0 + g:g0 + g + 1], g % 2, accum=True)
                    gather(xg[:], "T_qdcd", idx_sb["qdcdidx"][:, g0 + g:g0 + g + 1], g % 2, accum=True)
                    xgs.append(xg)
                    cg = gpool.tile([128, 3 * D], f32, tag=f"cg{g}")
                    gather(cg[:], "COMB", idx_sb["combidx"][:, g0 + g:g0 + g + 1], 2 + (g % 2))
                    cgs.append(cg)

                # ---- x^T via PE transpose -> psum -> sbuf ----
                xps = xppool.tile([128, 2 * D], f32, tag="xps")
                for g in range(2):
                    nc.tensor.transpose(out=xps[:, g * D:(g + 1) * D],
                                        in_=xgs[g][:], identity=ident[:])
                xT = xtpool.tile([128, 2 * D], f32, tag="xT")
                nc.vector.tensor_copy(xT[:], xps[:])

                # ---- flush previous chunk's boundary prod + y ----
                if pending is not None:
                    pprod, pct, pco, pca, ph, pnst, pk = pending
                    nc.gpsimd.tensor_tensor(out=pprod[:, (CH - 1) * 64:CH * 64],
                                            in0=ph[:], in1=xT[:, 0:64], op=Alu.mult)
                    nc.tensor.matmul(bass.AP(pct, pco + 1280, [[pca[0][0], 1], [1, 64 * pnst]]),
                                     ones_col[:], pprod[:, 0:64 * pnst],
                                     start=False, stop=True, skip_group_check=True)
                    ysb = spool.tile([1, 256], f32, tag="ysb")
                    nc.scalar.activation(ysb[:1, 0:64 * pnst],
                                         bass.AP(pct, pco + 1280, [[pca[0][0], 1], [1, 64 * pnst]]),
                                         Act.Sigmoid)
                    nc.sync.dma_start(out=t_y.ap()[pk * CH * BC: pk * CH * BC + 64 * pnst],
                                      in_=ysb[:1, 0:64 * pnst])
                    pending = None

                # ---- chunk psum: bankA = sdf1|sdf2', bankB = ki|pka1, bankC = pka2'|y ----
                cp = ppool.tile([128, 3 * 512], f32, tag="cp")
                base = cp[:]
                ct, co, ca = base.tensor, base.offset, base.ap

                def cps(col0, ncols):
                    return bass.AP(ct, co + col0, [[ca[0][0], 128], [1, ncols]])

                for g in range(2):   # ki bases -> bankB cols 0..255 (abs 512..767)
                    nc.tensor.matmul(cps(512 + g * 128, 128),
                                     cgs[g][:, 0:D], ident[:],
                                     start=(g == 0), stop=False,
                                     is_transpose=True, skip_group_check=True)
                for g in range(2):   # pka1 bases -> bankB cols 256..511
                    nc.tensor.matmul(cps(768 + g * 128, 128),
                                     cgs[g][:, D:2 * D], ident[:],
                                     start=False, stop=False,
                                     is_transpose=True, skip_group_check=True)
                for g in range(2):   # pka2' bases -> bankC cols 0..255
                    nc.tensor.matmul(cps(1024 + g * 128, 128),
                                     cgs[g][:, 2 * D:3 * D], ident[:],
                                     start=(g == 0), stop=False,
                                     is_transpose=True, skip_group_check=True)
                # x side of sdf gates -> bankA
                nc.tensor.matmul(cps(0, 256), w_sb["Wsdf1p"][:], xT[:],
                                 start=True, stop=False, skip_group_check=True)
                nc.tensor.matmul(cps(256, 256), w_sb["Wsdf2p2"][:], xT[:],
                                 start=False, stop=False, skip_group_check=True)

                prod = spool.tile([128, 256], f32, tag="prod")

                for s in range(nst):
                    nc.tensor.matmul(cps(0 + s * 64, 64), w_sb["W1n"][:], h[:],
                                     start=False, stop=False, skip_group_check=True)
                    nc.tensor.matmul(cps(256 + s * 64, 64), w_sb["W2n2"][:], h[:],
                                     start=False, stop=False, skip_group_check=True)
                    nc.tensor.matmul(cps(512 + s * 64, 64), w_sb["Wk1"][:], h[:],
                                     start=False, stop=False, skip_group_check=True)
                    gates1 = spool.tile([128, 192], f32, tag="gates1")
                    a1src = bass.AP(ct, co + s * 64, [[ca[0][0], 128], [256, 3], [1, 64]])
                    a1dst = gates1[:].rearrange("p (a b) -> p a b", b=64)
                    nc.scalar.activation(a1dst, a1src, Act.Sigmoid)
                    s1, s2p, gam = gates1[:, 0:64], gates1[:, 64:128], gates1[:, 128:192]
                    m = spool.tile([128, 64], f32, tag="m")
                    nc.vector.scalar_tensor_tensor(out=m[:], in0=s2p, scalar=2.0, in1=s1,
                                                   op0=Alu.mult, op1=Alu.mult)
                    # Wp@(m - s1) split as Wp@m + (-Wp)@s1: kills the sdf
                    # DVE hop on the critical path (s1 ready at act1 already)
                    nc.tensor.matmul(cps(768 + s * 64, 64), w_sb["Wp1N"][:], s1,
                                     start=False, stop=False, skip_group_check=True)
                    nc.tensor.matmul(cps(1024 + s * 64, 64), w_sb["Wp2N2"][:], s1,
                                     start=False, stop=False, skip_group_check=True)
                    nc.tensor.matmul(cps(768 + s * 64, 64), w_sb["Wp1"][:], m[:],
                                     start=False, stop=False, skip_group_check=True)
                    nc.tensor.matmul(cps(1024 + s * 64, 64), w_sb["Wp2x2"][:], m[:],
                                     start=False, stop=False, skip_group_check=True)
                    gates2 = spool.tile([128, 128], f32, tag="gates2")
                    a2src = bass.AP(ct, co + 768 + s * 64, [[ca[0][0], 128], [256, 2], [1, 64]])
                    a2dst = gates2[:].rearrange("p (a b) -> p a b", b=64)
                    nc.scalar.activation(a2dst, a2src, Act.Sigmoid)
                    p1, p2p = gates2[:, 0:64], gates2[:, 64:128]
                    m2 = spool.tile([128, 64], f32, tag="m2")
                    nc.vector.scalar_tensor_tensor(out=m2[:], in0=p2p, scalar=2.0, in1=p1,
                                                   op0=Alu.mult, op1=Alu.mult)
                    pka = spool.tile([128, 64], f32, tag="pka")
                    nc.vector.tensor_tensor(out=pka[:], in0=m2[:], in1=p1, op=Alu.subtract)
                    # h' = gam*h + (1-gam)*pka
                    gamc = spool.tile([128, 64], f32, tag="gamc")
                    nc.vector.tensor_scalar(out=gamc[:], in0=gam, scalar1=-1.0, scalar2=1.0,
                                            op0=Alu.mult, op1=Alu.add)
                    g1 = spool.tile([128, 64], f32, tag="g1")
                    nc.vector.tensor_tensor(out=g1[:], in0=gam, in1=h[:], op=Alu.mult)
                    u = spool.tile([128, 64], f32, tag="u")
                    nc.vector.tensor_tensor(out=u[:], in0=gamc[:], in1=pka[:], op=Alu.mult)
                    hn = hpool.tile([D, BC], f32, tag="h")
                    nc.vector.tensor_tensor(out=hn[:], in0=g1[:], in1=u[:], op=Alu.add)
                    h = hn
                    if s < nst - 1 or k == NCHUNK - 1:
                        nc.gpsimd.tensor_tensor(out=prod[:, s * 64:(s + 1) * 64],
                                                in0=h[:], in1=xT[:, (s + 1) * 64:(s + 2) * 64],
                                                op=Alu.mult)

                if k == NCHUNK - 1:
                    nc.tensor.matmul(bass.AP(ct, co + 1280, [[ca[0][0], 1], [1, 64 * nst]]),
                                     ones_col[:], prod[:, 0:64 * nst],
                                     start=False, stop=True, skip_group_check=True)
                    ysb = spool.tile([1, 256], f32, tag="ysb")
                    nc.scalar.activation(ysb[:1, 0:64 * nst],
                                         bass.AP(ct, co + 1280, [[ca[0][0], 1], [1, 64 * nst]]),
                                         Act.Sigmoid)
                    nc.sync.dma_start(out=t_y.ap()[k * CH * BC: k * CH * BC + 64 * nst],
                                      in_=ysb[:1, 0:64 * nst])
                else:
                    pending = (prod, ct, co, ca, h, nst, k)
    nc.compile()
    return nc


def kernel(**inputs):
    from concourse.bass_utils import run_bass_kernel_spmd

    w = _host_pack(**{k: np.asarray(inputs[k]) for k in
                      ["Eq", "Ec", "Eqd", "Ecd", "Ecorr", "Wx", "bx", "Wsdf1", "bsdf1",
                       "Wsdf2", "bsdf2", "Wpka1", "bpka1", "Wpka2", "bpka2", "Wki", "bki"]})
    q = np.asarray(inputs["question_seq"])
    c = np.asarray(inputs["concept_seq"])
    qd = np.asarray(inputs["question_diff_seq"])
    cd = np.asarray(inputs["concept_diff_seq"])
    co = np.asarray(inputs["correct_seq"])
    h0 = np.asarray(inputs["h0"], np.float32)
    qdcd = (qd * NCD + cd).astype(np.int64)
    comb = (qd * (NCD * 2) + cd * 2 + co).astype(np.int64)

    if "nc" not in _cache:
        _cache["nc"] = _build_program()
    nc = _cache["nc"]

    in_maps = []
    for core in range(NCORES):
        rows = slice(core * BC, (core + 1) * BC)
        m = dict(w)
        m["h0T"] = np.ascontiguousarray(h0[rows].T)
        m["qidx"] = _group_idx(q[rows].T)          # [S, BC] step-major
        m["cidx"] = _group_idx(c[rows].T)
        m["qdcdidx"] = _group_idx(qdcd[rows].T)
        m["combidx"] = _group_idx(comb[rows].T[:NSTEP])
        in_maps.append(m)

    global _last_in_maps
    _last_in_maps = in_maps
    res = run_bass_kernel_spmd(nc, in_maps, list(range(NCORES)))
    y = np.zeros((B, S), np.float32)
    for core in range(NCORES):
        yd = res.results[core]["y"][:NSTEP * BC].reshape(NSTEP, BC)
        y[core * BC:(core + 1) * BC, :NSTEP] = yd.T
    return y

